# revision 32
# baseline (speedup 1.0000x reference)
"""AttentionGate3D Trainium2 kernel.

Computes out = x * sigmoid(Wpsi @ relu(Wg@g + bg + Wx@x + bx) + bpsi) for
g, x of shape [2, 512, 32, 64, 64] (NCDHW); the convs are 1x1x1, i.e.
per-voxel channel matmuls.

Sharding: depth D=32 is split across the 8 NeuronCores (4 d-slices per
batch per core); 1x1x1 convs are per-voxel so no halo exchange is needed
and the (tiny) weights are replicated to every core.

Per-core layout is [B=2, C=512, V=16384] (channels x flat voxels).
The 16 per-core DMA engines are byte-rate-capped (~25 GB/s each, ~400
GB/s aggregate), so HBM bytes are the scarce resource:
  - g rides as fp8e4m3 (16.8 MB) and feeds fp8 DoubleRow matmuls (one
    instruction contracts 2x128 channels at the fp8 rate).
  - x rides as int8 with per-(batch, channel, 2048-voxel-block) scales
    (16.8 MB + 64 KB) and is dequantized on-chip to fp16 (scalar + DVE
    share the work); the fp16 copy feeds the gating multiply and half
    of the Wx conv, and is re-quantized (DVE) to fp8 for the other
    half, which runs as one DoubleRow matmul. Gating error <= scale/2
    ~ 0.013 absolute, well inside the error budget; out must stay fp16
    (33.5 MB).
HBM/core: 16.8(g) + 16.8(x) + 33.5(out) = 67.2 MB, putting the DMA
floor (~172 us) at the PE time (~180 us) — jointly bottlenecked.

Structure per 2048-voxel block (loads: g then x8 on the sync HWDGE
ring; stores + weight prefetch on the scalar ring):
  - dequant: per kc, x8[:, kc, :] * s[b,c,blk] -> fp16 (kc0 on the
    scalar engine, kc1-3 on the DVE), then kc0-1 fp16 -> fp8 on the
    DVE, pipelined a block ahead of the PE.
  - per 512-voxel chunk and 128-channel output block: 2 fp8 DoubleRow
    matmuls (Wg@g) + 1 fp8 DoubleRow (Wx@x, kc0-1) + 2 fp16 matmuls
    (Wx@x, kc2-3) share one PSUM accumulation, then a fused bias+ReLU
    on the scalar engine (PSUM -> fp16 SBUF).
  - psi matmuls use Wpsi replicated across all 128 stationary columns,
    so psi lands in PSUM already broadcast across partitions; they are
    emitted one chunk late so the PE never waits on the relu.
  - per 1024 voxels: fused bias+Sigmoid, then one DVE multiply
    out = x * attn with attn broadcast along the channel-chunk dim.
  - one 2 MB store per block (4 KB DMA rows; smaller stores drop the
    per-engine DMA rate); only the final block streams two 1 MB halves
    to shorten the drain after the last matmul.

Measured (8-core SPMD, all-core NTFF profiling): 204-228 us per core,
relative error 1.46e-2 vs the fp32 reference (gate 2e-2).
"""

import sys

sys.path.insert(0, "/opt/trn_rl_repo")

import ml_dtypes
import numpy as np

import concourse.bass as bass
import concourse.tile as tile
from concourse import bacc, mybir
from concourse.bass_utils import run_bass_kernel_spmd

N_CORES = 8
B, C, D, H, W = 2, 512, 32, 64, 64
F_INT = 256
D_PER_CORE = D // N_CORES           # 4
V = D_PER_CORE * H * W              # 16384 voxels per batch per core
VB = 2048                           # voxels per DMA block
CHUNK = 512                         # voxels per PSUM-bank matmul
KC = C // 128                       # 4 contraction chunks
KP = KC // 2                        # 2 DoubleRow pair-chunks for the g conv
MC = F_INT // 128                   # 2 output-channel chunks
PSI_KC = F_INT // 128               # 2 psi contraction chunks
N_VB = V // VB                      # 8 blocks per batch per core

F32 = mybir.dt.float32
F16 = mybir.dt.float16
F8 = mybir.dt.float8e4
I8 = mybir.dt.int8
NP_F8 = ml_dtypes.float8_e4m3

_cache: dict = {}


def _build(vb: int = VB, v: int = V):
    nc = bacc.Bacc(
        "TRN2", target_bir_lowering=False, debug=False, num_devices=N_CORES
    )

    n_vb = v // vb
    n_chunks = vb // CHUNK

    g_d = nc.declare_dram_parameter("g", [B, C, v], F8, isOutput=False)
    x8_d = nc.declare_dram_parameter("x8", [B, C, v], I8, isOutput=False)
    sc_d = nc.declare_dram_parameter("sc", [128, KC, B * n_vb], F32, isOutput=False)
    wg_d = nc.declare_dram_parameter("wg", [128, KC, F_INT], F8, isOutput=False)
    wx_d = nc.declare_dram_parameter("wx", [128, KC, F_INT], F16, isOutput=False)
    wx8_d = nc.declare_dram_parameter("wx8", [128, 2, F_INT], F8, isOutput=False)
    wpsi_d = nc.declare_dram_parameter("wpsi", [128, PSI_KC, 128], F16, isOutput=False)
    bgx_d = nc.declare_dram_parameter("bgx", [128, MC + 1], F32, isOutput=False)
    out_d = nc.declare_dram_parameter("out", [B, C, v], F16, isOutput=True)

    # [p, kc, v] views of the [B, C, v] tensors (channel c = kc*128 + p)
    g_v = g_d.rearrange("b (kc p) v -> b p kc v", p=128)
    x8_v = x8_d.rearrange("b (kc p) v -> b p kc v", p=128)
    out_v = out_d.rearrange("b (kc p) v -> b p kc v", p=128)

    with tile.TileContext(nc) as tc:
        with (
            tc.tile_pool(name="wpool", bufs=1) as wpool,
            tc.tile_pool(name="io", bufs=4) as io,
            tc.tile_pool(name="deq", bufs=3) as deq,
            tc.tile_pool(name="op", bufs=2) as op,
            tc.tile_pool(name="act", bufs=4) as actp,
            tc.tile_pool(name="psum", bufs=2, space="PSUM") as psum,
        ):
            wg_sb = wpool.tile([128, KC, F_INT], F8)
            wx_sb = wpool.tile([128, KC, F_INT], F16)
            wx8_sb = wpool.tile([128, 2, F_INT], F8)
            wpsi_sb = wpool.tile([128, PSI_KC, 128], F16)
            bgx_sb = wpool.tile([128, MC + 1], F32)
            sc_sb = wpool.tile([128, KC, B * n_vb], F32)
            # block-0 loads go first (x8 on the scalar ring AHEAD of the
            # weights, g on sync) so dequant and the PE start as early as
            # possible; weights follow on the scalar ring in consumption
            # order (sc -> dequant, wg -> first DoubleRow, ...)
            g0_t = io.tile([128, KC, vb], F8, tag="g")
            nc.sync.dma_start(g0_t[:], g_v[0, :, :, bass.ds(0, vb)])
            x80_t = io.tile([128, KC, vb], I8, tag="x8")
            nc.scalar.dma_start(x80_t[:], x8_v[0, :, :, bass.ds(0, vb)])
            nc.scalar.dma_start(sc_sb[:], sc_d[:])
            nc.scalar.dma_start(bgx_sb[:], bgx_d[:])
            nc.scalar.dma_start(wg_sb[:], wg_d[:])
            nc.scalar.dma_start(wx8_sb[:], wx8_d[:])
            nc.scalar.dma_start(wx_sb[:], wx_d[:])
            nc.scalar.dma_start(wpsi_sb[:], wpsi_d[:])

            # psi matmuls run one chunk late (so relu is long done when the
            # PE reaches them); sigmoid + gating multiply run per 2 chunks.
            pending = []               # [(relu tiles, chunk idx)]
            epilog = []                # [(psi psum, x tile, out tile, j0, width)]
            psi_state = [None, 0]      # (pspsi tile, base chunk), chunks filled

            def flush_epilog(split_store=None):
                # split_store = (b, block voxel offset) — used on the final
                # block only: store each 1024-voxel half as soon as it's
                # gated so the drain after the last matmul is one 1 MB store
                for (ps_psi, xh_t, o_t, j0, width) in epilog:
                    attn = actp.tile([128, 2 * CHUNK], F16, tag="attn")
                    nc.scalar.activation(
                        attn[:, :width],
                        ps_psi[:, :width],
                        mybir.ActivationFunctionType.Sigmoid,
                        bias=bgx_sb[:, MC : MC + 1],
                    )
                    vs2 = bass.ds(j0 * CHUNK, width)
                    nc.vector.tensor_mul(
                        o_t[:, :, vs2],
                        xh_t[:, :, vs2],
                        attn[:, :width].unsqueeze(1).to_broadcast([128, KC, width]),
                    )
                    if split_store is not None:
                        bb, v0 = split_store
                        nc.scalar.dma_start(
                            out_v[bb, :, :, bass.ds(v0 + j0 * CHUNK, width)],
                            o_t[:, :, vs2],
                        )
                epilog.clear()

            def flush_pending():
                for (relu_t, js) in pending:
                    if psi_state[0] is None:
                        ps_psi_new = psum.tile(
                            [128, 2 * CHUNK], F32, tag="pspsi", name="ps_psi"
                        )
                        psi_state[0] = (ps_psi_new, js)
                        psi_state[1] = 0
                    ps_psi, j0 = psi_state[0]
                    off = psi_state[1] * CHUNK
                    for m in range(PSI_KC):
                        nc.tensor.matmul(
                            ps_psi[:, off : off + CHUNK],
                            wpsi_sb[:, m, :],
                            relu_t[m][:],
                            start=(m == 0),
                            stop=(m == PSI_KC - 1),
                        )
                    psi_state[1] += 1
                pending.clear()

            def close_psi(xh_t, o_t):
                if psi_state[0] is not None:
                    ps_psi, j0 = psi_state[0]
                    epilog.append((ps_psi, xh_t, o_t, j0, psi_state[1] * CHUNK))
                    psi_state[0] = None
                    psi_state[1] = 0

            for b in range(B):
                for i in range(n_vb):
                    vs = bass.ds(i * vb, vb)
                    last = (b == B - 1) and (i == n_vb - 1)
                    # g and x8 share the sync HWDGE ring (g first — the PE
                    # consumes it first); the scalar ring carries the stores
                    if b == 0 and i == 0:
                        g_t, x8_t = g0_t, x80_t
                    else:
                        g_t = io.tile([128, KC, vb], F8, tag="g")
                        nc.sync.dma_start(g_t[:], g_v[b, :, :, vs])
                        x8_t = io.tile([128, KC, vb], I8, tag="x8")
                        nc.sync.dma_start(x8_t[:], x8_v[b, :, :, vs])

                    # dequant x8 -> fp16 (per-kc per-block scale); kc0 on the
                    # scalar engine, kc1-3 on the DVE; then kc0-1 fp16 -> fp8
                    # on the gpsimd engine for the DoubleRow half of Wx@x
                    xh_t = deq.tile([128, KC, vb], F16, tag="xh")
                    blk = b * n_vb + i
                    nc.scalar.activation(
                        xh_t[:, 0, :],
                        x8_t[:, 0, :],
                        mybir.ActivationFunctionType.Copy,
                        scale=sc_sb[:, 0, blk : blk + 1],
                    )
                    for kc in range(1, KC - 1):
                        nc.vector.tensor_scalar_mul(
                            xh_t[:, kc, :],
                            x8_t[:, kc, :],
                            sc_sb[:, kc, blk : blk + 1],
                        )
                    # kc3 dequant on the (otherwise idle) gpsimd engine
                    nc.gpsimd.tensor_scalar_mul(
                        xh_t[:, KC - 1, :],
                        x8_t[:, KC - 1, :],
                        sc_sb[:, KC - 1, blk : blk + 1],
                    )
                    xf8_t = deq.tile([128, 2, vb], F8, tag="xf8")
                    nc.vector.tensor_copy(xf8_t[:], xh_t[:, 0:2, :])
                    o_t = op.tile([128, KC, vb], F16, tag="o")

                    for j in range(n_chunks):
                        cs = bass.ts(j, CHUNK)
                        relu_t = []
                        for m in range(MC):
                            ps = psum.tile([128, CHUNK], F32, tag=f"ps{m}")
                            ms = bass.ts(m, 128)
                            for kp in range(KP):
                                nc.tensor.matmul(
                                    ps[:],
                                    wg_sb[:, 2 * kp : 2 * kp + 2, ms],
                                    g_t[:, 2 * kp : 2 * kp + 2, cs],
                                    start=(kp == 0),
                                    stop=False,
                                    perf_mode=mybir.MatmulPerfMode.DoubleRow,
                                )
                            nc.tensor.matmul(
                                ps[:],
                                wx8_sb[:, :, ms],
                                xf8_t[:, :, cs],
                                start=False,
                                stop=False,
                                perf_mode=mybir.MatmulPerfMode.DoubleRow,
                            )
                            for kc in range(2, KC):
                                nc.tensor.matmul(
                                    ps[:],
                                    wx_sb[:, kc, ms],
                                    xh_t[:, kc, cs],
                                    start=False,
                                    stop=(kc == KC - 1),
                                )
                            rt = actp.tile([128, CHUNK], F16, tag=f"relu{m}")
                            nc.scalar.activation(
                                rt[:],
                                ps[:],
                                mybir.ActivationFunctionType.Relu,
                                bias=bgx_sb[:, m : m + 1],
                            )
                            relu_t.append(rt)
                        flush_pending()
                        if j % 2 == 0 and j > 0:
                            close_psi(xh_t, o_t)
                            flush_epilog((b, i * vb) if last else None)
                        pending.append((relu_t, j))

                    flush_pending()
                    close_psi(xh_t, o_t)
                    if last:
                        flush_epilog((b, i * vb))
                    else:
                        flush_epilog()
                        nc.scalar.dma_start(out_v[b, :, :, vs], o_t[:])

    nc.compile()
    return nc


def _prep_weights(Wg, bg, Wx, bx, Wpsi, bpsi_val):
    # wg[p, kc, m] = Wg[m, kc*128 + p] (stationary lhsT chunks; DoubleRow
    # matmuls consume [:, 2k:2k+2, ms] pairs)
    wg = np.ascontiguousarray(
        Wg.T.reshape(KC, 128, F_INT).transpose(1, 0, 2)
    ).astype(NP_F8)
    wx = np.ascontiguousarray(
        Wx.T.reshape(KC, 128, F_INT).transpose(1, 0, 2)
    ).astype(np.float16)
    # fp8 copy of the kc0/kc1 half of Wx for the DoubleRow x matmul
    wx8 = np.ascontiguousarray(wx[:, 0:2, :]).astype(NP_F8)
    # wpsi[p, m_chunk, :] = Wpsi[0, m_chunk*128 + p], replicated across all
    # 128 stationary columns so psi lands broadcast across partitions.
    wp = Wpsi[0].reshape(PSI_KC, 128).T
    wpsi = np.ascontiguousarray(np.repeat(wp[:, :, None], 128, axis=2)).astype(
        np.float16
    )
    bgx = np.empty((128, MC + 1), dtype=np.float32)
    bgx[:, :MC] = (bg + bx).reshape(MC, 128).T
    bgx[:, MC] = bpsi_val
    return wg, wx, wx8, wpsi, bgx


def _quant_x(xc):
    # xc: [B, C, V] fp32 -> int8 with per-(b, c, 2048-voxel-block) scales.
    xb = xc.reshape(B, C, N_VB, VB)
    s = np.abs(xb).max(axis=3) / 127.0          # [B, C, N_VB]
    s = np.maximum(s, 1e-8)
    q = np.rint(xb / s[..., None]).astype(np.int8).reshape(B, C, V)
    # sc[p, kc, b*N_VB + i] = s[b, kc*128 + p, i]
    sc = np.ascontiguousarray(
        s.reshape(B, KC, 128, N_VB).transpose(2, 1, 0, 3).reshape(128, KC, B * N_VB)
    ).astype(np.float32)
    return q, sc


def kernel(g, x, Wg, bg, Wx, bx, Wpsi, bpsi, _trace=False):
    if "nc" not in _cache:
        _cache["nc"] = _build()
    nc = _cache["nc"]

    g = np.asarray(g, dtype=np.float32)
    x = np.asarray(x, dtype=np.float32)
    bpsi_val = float(np.asarray(bpsi).reshape(-1)[0])
    wg, wx, wx8, wpsi, bgx = _prep_weights(
        np.asarray(Wg, np.float32),
        np.asarray(bg, np.float32),
        np.asarray(Wx, np.float32),
        np.asarray(bx, np.float32),
        np.asarray(Wpsi, np.float32),
        bpsi_val,
    )
    in_maps = []
    for k in range(N_CORES):
        sl = slice(k * D_PER_CORE, (k + 1) * D_PER_CORE)
        xq, sc = _quant_x(
            np.ascontiguousarray(x[:, :, sl]).reshape(B, C, V)
        )
        in_maps.append(
            {
                "g": np.ascontiguousarray(g[:, :, sl])
                .reshape(B, C, V)
                .astype(NP_F8),
                "x8": xq,
                "sc": sc,
                "wg": wg,
                "wx": wx,
                "wx8": wx8,
                "wpsi": wpsi,
                "bgx": bgx,
            }
        )
    try:
        res = run_bass_kernel_spmd(nc, in_maps, list(range(N_CORES)), trace=_trace)
    except Exception:
        # transient axon/PJRT hiccups have been observed; one retry
        res = run_bass_kernel_spmd(nc, in_maps, list(range(N_CORES)), trace=_trace)

    out = np.empty((B, C, D, H, W), dtype=np.float32)
    for k in range(N_CORES):
        sl = slice(k * D_PER_CORE, (k + 1) * D_PER_CORE)
        out[:, :, sl] = (
            res.results[k]["out"].astype(np.float32).reshape(B, C, D_PER_CORE, H, W)
        )
    if _trace:
        return out, res
    return out


# revision 34
# speedup vs baseline: 2.9863x; 2.9863x over previous
"""AttentionGate3D Trainium2 kernel.

Computes out = x * sigmoid(Wpsi @ relu(Wg@g + bg + Wx@x + bx) + bpsi) for
g, x of shape [2, 512, 32, 64, 64] (NCDHW); the convs are 1x1x1, i.e.
per-voxel channel matmuls.

Sharding: depth D=32 is split across the 8 NeuronCores (4 d-slices per
batch per core); 1x1x1 convs are per-voxel so no halo exchange is needed
and the (tiny) weights are replicated to every core.

Per-core layout is [B=2, C=512, V=16384] (channels x flat voxels).
The 16 per-core DMA engines are byte-rate-capped (~25 GB/s each, ~400
GB/s aggregate), so HBM bytes are the scarce resource:
  - g rides as fp8e4m3 (16.8 MB) and feeds fp8 DoubleRow matmuls (one
    instruction contracts 2x128 channels at the fp8 rate).
  - x rides as int8 with per-(batch, channel, 2048-voxel-block) scales
    (16.8 MB + 64 KB) and is dequantized on-chip to fp16 (scalar + DVE
    share the work); the fp16 copy feeds the gating multiply and half
    of the Wx conv, and is re-quantized (DVE) to fp8 for the other
    half, which runs as one DoubleRow matmul. Gating error <= scale/2
    ~ 0.013 absolute, well inside the error budget; out must stay fp16
    (33.5 MB).
HBM/core: 16.8(g) + 16.8(x) + 33.5(out) = 67.2 MB, putting the DMA
floor (~172 us) at the PE time (~180 us) — jointly bottlenecked.

Structure per 2048-voxel block (loads: g then x8 on the sync HWDGE
ring; stores + weight prefetch on the scalar ring):
  - dequant: per kc, x8[:, kc, :] * s[b,c,blk] -> fp16 (kc0 on the
    scalar engine, kc1-3 on the DVE), then kc0-1 fp16 -> fp8 on the
    DVE, pipelined a block ahead of the PE.
  - per 512-voxel chunk and 128-channel output block: 2 fp8 DoubleRow
    matmuls (Wg@g) + 1 fp8 DoubleRow (Wx@x, kc0-1) + 2 fp16 matmuls
    (Wx@x, kc2-3) share one PSUM accumulation, then a fused bias+ReLU
    on the scalar engine (PSUM -> fp16 SBUF).
  - psi matmuls use Wpsi replicated across all 128 stationary columns,
    so psi lands in PSUM already broadcast across partitions; they are
    emitted one chunk late so the PE never waits on the relu.
  - per 1024 voxels: fused bias+Sigmoid, then one DVE multiply
    out = x * attn with attn broadcast along the channel-chunk dim.
  - one 2 MB store per block (4 KB DMA rows; smaller stores drop the
    per-engine DMA rate); only the final block streams two 1 MB halves
    to shorten the drain after the last matmul.

Measured (8-core SPMD, all-core NTFF profiling): 204-228 us per core,
relative error 1.46e-2 vs the fp32 reference (gate 2e-2).
"""

import sys

sys.path.insert(0, "/opt/trn_rl_repo")

import ml_dtypes
import numpy as np

import concourse.bass as bass
import concourse.tile as tile
from concourse import bacc, mybir
from concourse.bass_utils import run_bass_kernel_spmd

N_CORES = 8
B, C, D, H, W = 2, 512, 32, 64, 64
F_INT = 256
D_PER_CORE = D // N_CORES           # 4
V = D_PER_CORE * H * W              # 16384 voxels per batch per core
VB = 2048                           # voxels per DMA block
CHUNK = 512                         # voxels per PSUM-bank matmul
KC = C // 128                       # 4 contraction chunks
KP = KC // 2                        # 2 DoubleRow pair-chunks for the g conv
MC = F_INT // 128                   # 2 output-channel chunks
PSI_KC = F_INT // 128               # 2 psi contraction chunks
N_VB = V // VB                      # 8 blocks per batch per core

F32 = mybir.dt.float32
F16 = mybir.dt.float16
F8 = mybir.dt.float8e4
I8 = mybir.dt.int8
NP_F8 = ml_dtypes.float8_e4m3

_cache: dict = {}


def _build(vb: int = VB, v: int = V):
    nc = bacc.Bacc(
        "TRN2", target_bir_lowering=False, debug=False, num_devices=N_CORES
    )

    n_vb = v // vb
    n_chunks = vb // CHUNK

    g_d = nc.declare_dram_parameter("g", [B, C, v], F8, isOutput=False)
    x8_d = nc.declare_dram_parameter("x8", [B, C, v], I8, isOutput=False)
    sc_d = nc.declare_dram_parameter("sc", [128, KC, B * n_vb], F32, isOutput=False)
    wg_d = nc.declare_dram_parameter("wg", [128, KC, F_INT], F8, isOutput=False)
    wx_d = nc.declare_dram_parameter("wx", [128, KC, F_INT], F16, isOutput=False)
    wx8_d = nc.declare_dram_parameter("wx8", [128, 2, F_INT], F8, isOutput=False)
    wpsi_d = nc.declare_dram_parameter("wpsi", [128, PSI_KC, 128], F16, isOutput=False)
    bgx_d = nc.declare_dram_parameter("bgx", [128, MC + 1], F32, isOutput=False)
    out_d = nc.declare_dram_parameter("out", [B, C, v], F16, isOutput=True)

    # [p, kc, v] views of the [B, C, v] tensors (channel c = kc*128 + p)
    g_v = g_d.rearrange("b (kc p) v -> b p kc v", p=128)
    x8_v = x8_d.rearrange("b (kc p) v -> b p kc v", p=128)
    out_v = out_d.rearrange("b (kc p) v -> b p kc v", p=128)

    with tile.TileContext(nc) as tc:
        with (
            tc.tile_pool(name="wpool", bufs=1) as wpool,
            tc.tile_pool(name="io", bufs=4) as io,
            tc.tile_pool(name="deq", bufs=3) as deq,
            tc.tile_pool(name="op", bufs=2) as op,
            tc.tile_pool(name="act", bufs=4) as actp,
            tc.tile_pool(name="psum", bufs=2, space="PSUM") as psum,
        ):
            wg_sb = wpool.tile([128, KC, F_INT], F8)
            wx_sb = wpool.tile([128, KC, F_INT], F16)
            wx8_sb = wpool.tile([128, 2, F_INT], F8)
            wpsi_sb = wpool.tile([128, PSI_KC, 128], F16)
            bgx_sb = wpool.tile([128, MC + 1], F32)
            sc_sb = wpool.tile([128, KC, B * n_vb], F32)
            # block-0 loads go first (x8 on the scalar ring AHEAD of the
            # weights, g on sync) so dequant and the PE start as early as
            # possible; weights follow on the scalar ring in consumption
            # order (sc -> dequant, wg -> first DoubleRow, ...)
            g0_t = io.tile([128, KC, vb], F8, tag="g")
            nc.sync.dma_start(g0_t[:], g_v[0, :, :, bass.ds(0, vb)])
            x80_t = io.tile([128, KC, vb], I8, tag="x8")
            nc.scalar.dma_start(x80_t[:], x8_v[0, :, :, bass.ds(0, vb)])
            nc.scalar.dma_start(sc_sb[:], sc_d[:])
            nc.scalar.dma_start(bgx_sb[:], bgx_d[:])
            nc.scalar.dma_start(wg_sb[:], wg_d[:])
            nc.scalar.dma_start(wx8_sb[:], wx8_d[:])
            nc.scalar.dma_start(wx_sb[:], wx_d[:])
            nc.scalar.dma_start(wpsi_sb[:], wpsi_d[:])

            # psi matmuls run one chunk late (so relu is long done when the
            # PE reaches them); sigmoid + gating multiply run per 2 chunks.
            pending = []               # [(relu tiles, chunk idx)]
            epilog = []                # [(psi psum, x tile, out tile, j0, width)]
            psi_state = [None, 0]      # (pspsi tile, base chunk), chunks filled

            def flush_epilog(split_store=None):
                # split_store = (b, block voxel offset) — used on the final
                # block only: store each 1024-voxel half as soon as it's
                # gated so the drain after the last matmul is one 1 MB store
                for (ps_psi, xh_t, o_t, j0, width) in epilog:
                    attn = actp.tile([128, 2 * CHUNK], F16, tag="attn")
                    nc.scalar.activation(
                        attn[:, :width],
                        ps_psi[:, :width],
                        mybir.ActivationFunctionType.Sigmoid,
                        bias=bgx_sb[:, MC : MC + 1],
                    )
                    vs2 = bass.ds(j0 * CHUNK, width)
                    nc.vector.tensor_mul(
                        o_t[:, :, vs2],
                        xh_t[:, :, vs2],
                        attn[:, :width].unsqueeze(1).to_broadcast([128, KC, width]),
                    )
                    if split_store is not None:
                        bb, v0 = split_store
                        nc.scalar.dma_start(
                            out_v[bb, :, :, bass.ds(v0 + j0 * CHUNK, width)],
                            o_t[:, :, vs2],
                        )
                epilog.clear()

            def flush_pending():
                for (relu_t, js) in pending:
                    if psi_state[0] is None:
                        ps_psi_new = psum.tile(
                            [128, 2 * CHUNK], F32, tag="pspsi", name="ps_psi"
                        )
                        psi_state[0] = (ps_psi_new, js)
                        psi_state[1] = 0
                    ps_psi, j0 = psi_state[0]
                    off = psi_state[1] * CHUNK
                    for m in range(PSI_KC):
                        nc.tensor.matmul(
                            ps_psi[:, off : off + CHUNK],
                            wpsi_sb[:, m, :],
                            relu_t[m][:],
                            start=(m == 0),
                            stop=(m == PSI_KC - 1),
                        )
                    psi_state[1] += 1
                pending.clear()

            def close_psi(xh_t, o_t):
                if psi_state[0] is not None:
                    ps_psi, j0 = psi_state[0]
                    epilog.append((ps_psi, xh_t, o_t, j0, psi_state[1] * CHUNK))
                    psi_state[0] = None
                    psi_state[1] = 0

            for b in range(B):
                for i in range(n_vb):
                    vs = bass.ds(i * vb, vb)
                    last = (b == B - 1) and (i == n_vb - 1)
                    # g and x8 share the sync HWDGE ring (g first — the PE
                    # consumes it first); the scalar ring carries the stores
                    if b == 0 and i == 0:
                        g_t, x8_t = g0_t, x80_t
                    else:
                        g_t = io.tile([128, KC, vb], F8, tag="g")
                        nc.sync.dma_start(g_t[:], g_v[b, :, :, vs])
                        x8_t = io.tile([128, KC, vb], I8, tag="x8")
                        nc.sync.dma_start(x8_t[:], x8_v[b, :, :, vs])

                    # dequant x8 -> fp16 (per-kc per-block scale); kc0 on the
                    # scalar engine, kc1-3 on the DVE; then kc0-1 fp16 -> fp8
                    # on the gpsimd engine for the DoubleRow half of Wx@x
                    xh_t = deq.tile([128, KC, vb], F16, tag="xh")
                    blk = b * n_vb + i
                    nc.scalar.activation(
                        xh_t[:, 0, :],
                        x8_t[:, 0, :],
                        mybir.ActivationFunctionType.Copy,
                        scale=sc_sb[:, 0, blk : blk + 1],
                    )
                    # the scale is shared across kc (per (b, partition, block)
                    # group-max quantization), so kc1-3 dequant in ONE DVE
                    # call — fewer instructions and semaphore hops
                    nc.vector.tensor_scalar_mul(
                        xh_t[:, 1:KC, :],
                        x8_t[:, 1:KC, :],
                        sc_sb[:, 1, blk : blk + 1],
                    )
                    xf8_t = deq.tile([128, 2, vb], F8, tag="xf8")
                    nc.vector.tensor_copy(xf8_t[:], xh_t[:, 0:2, :])
                    o_t = op.tile([128, KC, vb], F16, tag="o")

                    for j in range(n_chunks):
                        cs = bass.ts(j, CHUNK)
                        relu_t = []
                        for m in range(MC):
                            ps = psum.tile([128, CHUNK], F32, tag=f"ps{m}")
                            ms = bass.ts(m, 128)
                            for kp in range(KP):
                                nc.tensor.matmul(
                                    ps[:],
                                    wg_sb[:, 2 * kp : 2 * kp + 2, ms],
                                    g_t[:, 2 * kp : 2 * kp + 2, cs],
                                    start=(kp == 0),
                                    stop=False,
                                    perf_mode=mybir.MatmulPerfMode.DoubleRow,
                                )
                            nc.tensor.matmul(
                                ps[:],
                                wx8_sb[:, :, ms],
                                xf8_t[:, :, cs],
                                start=False,
                                stop=False,
                                perf_mode=mybir.MatmulPerfMode.DoubleRow,
                            )
                            for kc in range(2, KC):
                                nc.tensor.matmul(
                                    ps[:],
                                    wx_sb[:, kc, ms],
                                    xh_t[:, kc, cs],
                                    start=False,
                                    stop=(kc == KC - 1),
                                )
                            rt = actp.tile([128, CHUNK], F16, tag=f"relu{m}")
                            nc.scalar.activation(
                                rt[:],
                                ps[:],
                                mybir.ActivationFunctionType.Relu,
                                bias=bgx_sb[:, m : m + 1],
                            )
                            relu_t.append(rt)
                        flush_pending()
                        if j % 2 == 0 and j > 0:
                            close_psi(xh_t, o_t)
                            flush_epilog((b, i * vb) if last else None)
                        pending.append((relu_t, j))

                    flush_pending()
                    close_psi(xh_t, o_t)
                    if last:
                        flush_epilog((b, i * vb))
                    else:
                        flush_epilog()
                        nc.scalar.dma_start(out_v[b, :, :, vs], o_t[:])

    nc.compile()
    return nc


def _prep_weights(Wg, bg, Wx, bx, Wpsi, bpsi_val):
    # wg[p, kc, m] = Wg[m, kc*128 + p] (stationary lhsT chunks; DoubleRow
    # matmuls consume [:, 2k:2k+2, ms] pairs)
    wg = np.ascontiguousarray(
        Wg.T.reshape(KC, 128, F_INT).transpose(1, 0, 2)
    ).astype(NP_F8)
    wx = np.ascontiguousarray(
        Wx.T.reshape(KC, 128, F_INT).transpose(1, 0, 2)
    ).astype(np.float16)
    # fp8 copy of the kc0/kc1 half of Wx for the DoubleRow x matmul
    wx8 = np.ascontiguousarray(wx[:, 0:2, :]).astype(NP_F8)
    # wpsi[p, m_chunk, :] = Wpsi[0, m_chunk*128 + p], replicated across all
    # 128 stationary columns so psi lands broadcast across partitions.
    wp = Wpsi[0].reshape(PSI_KC, 128).T
    wpsi = np.ascontiguousarray(np.repeat(wp[:, :, None], 128, axis=2)).astype(
        np.float16
    )
    bgx = np.empty((128, MC + 1), dtype=np.float32)
    bgx[:, :MC] = (bg + bx).reshape(MC, 128).T
    bgx[:, MC] = bpsi_val
    return wg, wx, wx8, wpsi, bgx


def _quant_x(xc):
    # xc: [B, C, V] fp32 -> int8 with per-(b, partition, 2048-voxel-block)
    # scales shared by the 4 channels {p, p+128, p+256, p+384} so the
    # on-chip dequant needs one scale vector per block.
    xb = xc.reshape(B, KC, 128, N_VB, VB)
    s = np.abs(xb).max(axis=(1, 4)) / 127.0     # [B, 128, N_VB]
    s = np.maximum(s, 1e-8)
    q = (
        np.rint(xb / s[:, None, :, :, None])
        .astype(np.int8)
        .reshape(B, C, V)
    )
    # sc[p, kc, b*N_VB + i] = s[b, p, i]  (equal across kc)
    sc = np.ascontiguousarray(
        np.broadcast_to(
            s.transpose(1, 0, 2).reshape(128, 1, B * N_VB), (128, KC, B * N_VB)
        )
    ).astype(np.float32)
    return q, sc


def kernel(g, x, Wg, bg, Wx, bx, Wpsi, bpsi, _trace=False):
    if "nc" not in _cache:
        _cache["nc"] = _build()
    nc = _cache["nc"]

    g = np.asarray(g, dtype=np.float32)
    x = np.asarray(x, dtype=np.float32)
    bpsi_val = float(np.asarray(bpsi).reshape(-1)[0])
    wg, wx, wx8, wpsi, bgx = _prep_weights(
        np.asarray(Wg, np.float32),
        np.asarray(bg, np.float32),
        np.asarray(Wx, np.float32),
        np.asarray(bx, np.float32),
        np.asarray(Wpsi, np.float32),
        bpsi_val,
    )
    in_maps = []
    for k in range(N_CORES):
        sl = slice(k * D_PER_CORE, (k + 1) * D_PER_CORE)
        xq, sc = _quant_x(
            np.ascontiguousarray(x[:, :, sl]).reshape(B, C, V)
        )
        in_maps.append(
            {
                "g": np.ascontiguousarray(g[:, :, sl])
                .reshape(B, C, V)
                .astype(NP_F8),
                "x8": xq,
                "sc": sc,
                "wg": wg,
                "wx": wx,
                "wx8": wx8,
                "wpsi": wpsi,
                "bgx": bgx,
            }
        )
    try:
        res = run_bass_kernel_spmd(nc, in_maps, list(range(N_CORES)), trace=_trace)
    except Exception:
        # transient axon/PJRT hiccups have been observed; one retry
        res = run_bass_kernel_spmd(nc, in_maps, list(range(N_CORES)), trace=_trace)

    out = np.empty((B, C, D, H, W), dtype=np.float32)
    for k in range(N_CORES):
        sl = slice(k * D_PER_CORE, (k + 1) * D_PER_CORE)
        out[:, :, sl] = (
            res.results[k]["out"].astype(np.float32).reshape(B, C, D_PER_CORE, H, W)
        )
    if _trace:
        return out, res
    return out
